# revision 14
# baseline (speedup 1.0000x reference)
"""Trainium2 Bass kernel for CompositionalGatedRecurrence (v2).

Strategy (8 cores = batch x seq-half; ROWS=1024 rows/core, full hidden dim)
--------------------------------------------------------------------------
Same decomposition as v1, rebuilt around three findings from the v1 trace
and a calibrated host-side error model:

* fp16 replaces bf16 everywhere (same PE/DVE throughput, ~8x lower rounding
  error: model floor 8.5e-4 vs 6.5e-3).  The freed error budget pays for
  fully-fp8 g AND og banks (sigmoid compresses the quantization noise);
  fp8 weights are pre-scaled on host (x64) into e4m3's normal range and
  descaled for free via the SIG activation's scale operand.
* The AllReduce takes ~32us end-to-end, so the post-AR correction chain is
  kept off every engine queue that has pre-AR work: stc/om run on GpSimd
  (otherwise idle), stc2/sq on Vector behind the tq backlog, q^2 on Scalar.
  om tiles therefore trickle out right after the collective lands while the
  PE is still in the q bank.
* og and q banks interleave per-ht (ogs needs only 3 buffers), and the
  projection's first pass is 7 PSUM banks wide consuming om tiles in
  kt-availability order, with the fp16 sum-of-squares matmuls slotted into
  the bubbles; ss owns the 8th PSUM bank (own pool) so the rstd -> fin
  dependency can never deadlock the pj bank rotation.
* sq = (64 q)^2 * stc^2 keeps the tiny out^2 values (~1e-4) in fp16's
  normal range; the 4096x is folded into the rstd constants exactly.
* Warm-up is 6 junk fp8 matmuls covering the HW p-state ramp
  (0.65->2.4GHz over ~4us) while the first weight/activation chunks stream.
"""

import numpy as np
import ml_dtypes

F16NP = np.float16
F8NP = ml_dtypes.float8_e4m3

B, S, D = 4, 2048, 1024
H, DH = 16, 64
HID = 1024
NPRIM, RANK = 16, 256
NCORES = 8
ROWS = S // 2          # rows per core
DT = D // 128          # 8 d-model tiles
HT = HID // 128        # 8 hidden tiles
NR = ROWS // 512       # 2 row column-blocks for matmul N
EPS = float(np.finfo(np.float32).eps)
SQS = 64.0             # out is scaled by SQS inside the sum-of-squares path
HID_EPS_S = float(HID * EPS * SQS * SQS)
HID_S = float(HID * SQS * SQS)
S_G = 64.0             # host pre-scale for the fp8 g-bank weights
S_OG = 64.0            # host pre-scale for the fp8 og weights

_BUILT = {}


def _build():
    import contextlib
    import concourse.tile as tile
    from concourse import mybir, bacc

    F32 = mybir.dt.float32
    F16 = mybir.dt.float16
    F8 = mybir.dt.float8e4
    MULT = mybir.AluOpType.mult
    ADD = mybir.AluOpType.add
    SIG = mybir.ActivationFunctionType.Sigmoid
    SQRT = mybir.ActivationFunctionType.Sqrt
    SQUARE = mybir.ActivationFunctionType.Square
    COPY = mybir.ActivationFunctionType.Copy
    DR = mybir.MatmulPerfMode.DoubleRow

    nc = bacc.Bacc()

    # ---- DRAM parameters (per-core shards, host-linearized) ---------------
    xt = nc.declare_dram_parameter('xt', [128, DT, ROWS], F16, isOutput=False)
    x8t = nc.declare_dram_parameter('x8', [128, DT, ROWS], F8, isOutput=False)
    wg8t = nc.declare_dram_parameter('wg8', [128, DT, HID], F8, isOutput=False)
    wkt = nc.declare_dram_parameter('wk', [128, DT, HID], F16, isOutput=False)
    wvt = nc.declare_dram_parameter('wv', [128, DT, HID], F16, isOutput=False)
    wqt = nc.declare_dram_parameter('wq', [128, DT, HID], F16, isOutput=False)
    og8t = nc.declare_dram_parameter('og8', [128, DT, HID], F8, isOutput=False)
    opw = nc.declare_dram_parameter('opw', [128, HT, D], F16, isOutput=False)
    a_t = nc.declare_dram_parameter('a_t', [H, ROWS], F32, isOutput=False)
    ca_t = nc.declare_dram_parameter('ca_t', [H, ROWS], F16, isOutput=False)
    mc = nc.declare_dram_parameter('mc', [128, 1], F32, isOutput=False)
    ma = nc.declare_dram_parameter('ma', [128, 1], F32, isOutput=False)
    out_d = nc.declare_dram_parameter('out', [ROWS, D], F16, isOutput=True)

    with tile.TileContext(nc, pool_alloc_mode='queue') as tc, \
            contextlib.ExitStack() as ctx:
        p_res = ctx.enter_context(tc.tile_pool(name='res', bufs=1))
        p_ps = ctx.enter_context(tc.tile_pool(name='ps', bufs=7, space='PSUM'))
        p_ssp = ctx.enter_context(tc.tile_pool(name='ssp', bufs=1,
                                               space='PSUM'))
        p_dram = ctx.enter_context(tc.tile_pool(name='dram', bufs=1,
                                                space='DRAM'))

        # ---- whole-kernel residents --------------------------------------
        x_sb = p_res.tile([128, DT, ROWS], F16)
        x8_sb = p_res.tile([128, DT, ROWS], F8)
        opw_sb = p_res.tile([128, HT, D], F16)
        mc_sb = p_res.tile([128, 1], F32)
        ma_sb = p_res.tile([128, 1], F32)
        ones_sb = p_res.tile([128, 1], F16)
        s_eff = p_res.tile([128, HT], F32)
        contrib = p_res.tile([128, HT], F32)
        s_init = p_res.tile([128, HT], F32)
        st_tiles = [p_res.tile([128, ROWS], F16, name=f'st_{ht}')
                    for ht in range(HT)]

        nc.vector.memset(ones_sb, 1.0)

        # ---- PE warm-up: 6 junk matmuls cover the p-state ramp while the
        # first weight/activation chunks stream in
        junk = p_res.tile([128, 512], F16)
        nc.vector.memset(junk, 0.0)
        warm_sb = p_res.tile([1, 512], F16)
        wps = p_ps.tile([1, 512], F32, tag='ps', name='warm')
        for i in range(6):
            nc.tensor.matmul(wps, lhsT=ones_sb, rhs=junk,
                             start=(i == 0), stop=(i == 5))
        nc.scalar.activation(warm_sb, wps, COPY)

        def bcast2(dst, src, ht):
            """Broadcast the two head rows of tile ht to 64 lanes each."""
            nc.sync.dma_start(
                out=dst[0:64, :],
                in_=src[2 * ht:2 * ht + 1, :].to_broadcast([64, ROWS]))
            nc.sync.dma_start(
                out=dst[64:128, :],
                in_=src[2 * ht + 1:2 * ht + 2, :].to_broadcast([64, ROWS]))

        # =========== phase 1: banks k, g, v with progressive kv fuse ======
        with tc.tile_pool(name='bank', bufs=1) as p_bank:
            with tc.tile_pool(name='fuse', bufs=1) as p_fuse, \
                    tc.tile_pool(name='abp', bufs=1) as p_ab:
                w_k = p_bank.tile([128, DT, HID], F16, tag='w', bufs=2,
                                  name='w_k')
                wg8 = p_fuse.tile([128, DT, HID], F8, name='wg8')
                # k-bank chunks first (pace the dt-outer start), then the fp8
                # g inputs, then the later banks in need order
                for dt in range(DT):
                    nc.sync.dma_start(out=x_sb[:, dt, :], in_=xt[:, dt, :])
                    nc.sync.dma_start(out=w_k[:, dt, :], in_=wkt[:, dt, :])
                for dt in range(0, DT, 2):
                    nc.sync.dma_start(out=x8_sb[:, dt:dt + 2, :],
                                      in_=x8t[:, dt:dt + 2, :])
                    nc.sync.dma_start(out=wg8[:, dt:dt + 2, :],
                                      in_=wg8t[:, dt:dt + 2, :])
                nc.sync.dma_start(out=mc_sb, in_=mc[:, :])
                nc.sync.dma_start(out=ma_sb, in_=ma[:, :])

                # k bank: 4 sets of (nr, ht-half) so only 4 PSUM banks live
                k_sb = [p_fuse.tile([128, ROWS], F16, tag='kt', bufs=8,
                                    name=f'k_{ht}') for ht in range(HT)]
                for nr in range(NR):
                    for hh in range(2):
                        hts = range(4 * hh, 4 * hh + 4)
                        pss = {ht: p_ps.tile([128, 512], F32, tag='ps',
                                             name=f'yk_{ht}_{nr}')
                               for ht in hts}
                        for dt in range(DT):
                            for ht in hts:
                                nc.tensor.matmul(
                                    pss[ht],
                                    lhsT=w_k[:, dt, ht * 128:(ht + 1) * 128],
                                    rhs=x_sb[:, dt, nr * 512:(nr + 1) * 512],
                                    start=(dt == 0), stop=(dt == DT - 1))
                        for ht in hts:
                            nc.scalar.activation(
                                k_sb[ht][:, nr * 512:(nr + 1) * 512],
                                pss[ht], COPY)
                # later-bank weight streams (after the k-critical chunks)
                w_v = p_bank.tile([128, DT, HID], F16, tag='w', bufs=2,
                                  name='w_v')
                nc.sync.dma_start(out=w_v, in_=wvt[:, :, :])
                og8 = p_bank.tile([128, DT, HID], F8, tag='og8', name='og8')
                nc.sync.dma_start(out=og8, in_=og8t[:, :, :])
                nc.sync.dma_start(out=opw_sb, in_=opw[:, :, :])
                w_q = p_bank.tile([128, DT, HID], F16, tag='w', bufs=2,
                                  name='w_q')
                nc.sync.dma_start(out=w_q, in_=wqt[:, :, :])

                # g bank: fp8 DR -> sigmoid only; the tgk muls move into
                # the v loop so the Vector queue processes each ht's
                # tgk/kv/scan as one group (the scan chain -- and thus the
                # collective trigger -- ends ~5us earlier)
                sg_tiles = []
                for ht in range(HT):
                    sgs = []
                    for nr in range(NR):
                        sl = slice(nr * 512, (nr + 1) * 512)
                        ps = p_ps.tile([128, 512], F32, tag='ps',
                                       name=f'yg_{ht}_{nr}')
                        for dt in range(0, DT, 2):
                            nc.tensor.matmul(
                                ps,
                                lhsT=wg8[:, dt:dt + 2,
                                         ht * 128:(ht + 1) * 128],
                                rhs=x8_sb[:, dt:dt + 2, sl],
                                start=(dt == 0), stop=(dt == DT - 2),
                                perf_mode=DR)
                        sg_t = p_fuse.tile([128, 512], F16, tag='sg',
                                           bufs=16, name=f'sg_{ht}_{nr}')
                        nc.scalar.activation(sg_t, ps, SIG, scale=1.0 / S_G)
                        sgs.append(sg_t)
                    sg_tiles.append(sgs)

                # v bank + scan (zero-init local states); contrib on the
                # otherwise-idle GpSimd so Scalar never backlogs into og
                for ht in range(HT):
                    tk_t = p_fuse.tile([128, ROWS], F16, tag='tgk', bufs=2,
                                       name=f'tgk_{ht}')
                    kv_t = p_fuse.tile([128, ROWS], F32, tag='kv', bufs=3,
                                       name=f'kv_{ht}')
                    for nr in range(NR):
                        sl = slice(nr * 512, (nr + 1) * 512)
                        nc.vector.tensor_mul(tk_t[:, sl], k_sb[ht][:, sl],
                                             sg_tiles[ht][nr])
                    for nr in range(NR):
                        sl = slice(nr * 512, (nr + 1) * 512)
                        ps = p_ps.tile([128, 512], F32, tag='ps',
                                       name=f'yv_{ht}_{nr}')
                        for dt in range(DT):
                            nc.tensor.matmul(
                                ps,
                                lhsT=w_v[:, dt, ht * 128:(ht + 1) * 128],
                                rhs=x_sb[:, dt, sl],
                                start=(dt == 0), stop=(dt == DT - 1))
                        nc.vector.tensor_mul(kv_t[:, sl], ps, tk_t[:, sl])
                    ab_t = p_ab.tile([128, ROWS], F32, tag='ab', bufs=2,
                                     name=f'ab_{ht}')
                    bcast2(ab_t, a_t, ht)
                    nc.vector.tensor_tensor_scan(
                        st_tiles[ht], ab_t, kv_t, 0.0, MULT, ADD)
                    nc.gpsimd.tensor_scalar_mul(contrib[:, ht:ht + 1],
                                                st_tiles[ht][:, ROWS - 1:ROWS],
                                                mc_sb)

                # ---- boundary state exchange (pairs) ---------------------
                cin = p_dram.tile([128, HT], F32)
                cout = p_dram.tile([128, HT], F32)
                nc.sync.dma_start(out=cin, in_=contrib)
                nc.gpsimd.collective_compute(
                    'AllReduce', ADD,
                    replica_groups=[[0, 1], [2, 3], [4, 5], [6, 7]],
                    ins=[cin.opt()], outs=[cout.opt()])
            # p_fuse / p_ab closed: wg8, k, tgk, sg, kv, ab freed

            # =========== phase 2: og+q interleaved, post-AR chain =========
            with tc.tile_pool(name='post', bufs=1) as p_post:
                # cab prefetch; the s_init DMA is slotted after cab2 so it
                # neither blocks the first correction tiles nor queues
                # behind a buffer-limited broadcast
                cab_tiles = []
                for ht in range(3):
                    cab_t = p_post.tile([128, ROWS], F16, tag='cab', bufs=3,
                                        name=f'cab_{ht}')
                    bcast2(cab_t, ca_t, ht)
                    cab_tiles.append(cab_t)
                nc.sync.dma_start(out=s_init, in_=cout)
                for ht in range(3, HT):
                    cab_t = p_post.tile([128, ROWS], F16, tag='cab', bufs=3,
                                        name=f'cab_{ht}')
                    bcast2(cab_t, ca_t, ht)
                    cab_tiles.append(cab_t)
                # s_eff heads the (empty) phase-2 Vector queue
                nc.vector.tensor_scalar_mul(s_eff, s_init, ma_sb)

                # og + q interleaved per ht; both drain on Scalar so the
                # Vector queue stays EMPTY in phase 2 until the post-AR
                # chain -- nothing can head-of-line block the correction
                ogs_tiles = []
                q_tiles = []
                for ht in range(HT):
                    ogs_t = p_post.tile([128, ROWS], F16, tag='ogs', bufs=3,
                                        name=f'ogs_{ht}')
                    for nr in range(NR):
                        sl = slice(nr * 512, (nr + 1) * 512)
                        ps = p_ps.tile([128, 512], F32, tag='ps',
                                       name=f'yo_{ht}_{nr}')
                        for dt in range(0, DT, 2):
                            nc.tensor.matmul(
                                ps,
                                lhsT=og8[:, dt:dt + 2,
                                         ht * 128:(ht + 1) * 128],
                                rhs=x8_sb[:, dt:dt + 2, sl],
                                start=(dt == 0), stop=(dt == DT - 2),
                                perf_mode=DR)
                        nc.scalar.activation(ogs_t[:, sl], ps, SIG,
                                             scale=1.0 / S_OG)
                    qog_t = p_post.tile([128, ROWS], F16, tag='qog',
                                        bufs=8, name=f'qog_{ht}')
                    q2_t = p_post.tile([128, ROWS], F16, tag='q2', bufs=8,
                                       name=f'q2_{ht}')
                    for nr in range(NR):
                        sl = slice(nr * 512, (nr + 1) * 512)
                        ps = p_ps.tile([128, 512], F32, tag='ps',
                                       name=f'yq_{ht}_{nr}')
                        for dt in range(DT):
                            nc.tensor.matmul(
                                ps,
                                lhsT=w_q[:, dt, ht * 128:(ht + 1) * 128],
                                rhs=x_sb[:, dt, sl],
                                start=(dt == 0), stop=(dt == DT - 1))
                        nc.vector.tensor_mul(qog_t[:, sl], ps, ogs_t[:, sl])
                        nc.scalar.activation(q2_t[:, sl], ps, SQUARE,
                                             scale=SQS)
                    ogs_tiles.append(ogs_t)
                    q_tiles.append((qog_t, q2_t))

                # post-AR chain: Vector runs stc -> om at ~2us/tile the
                # moment the collective lands (om = qog*stc, qog pre-built
                # from the q psums); Scalar squares stc behind it, and the
                # sq = q2*stc2 block follows on Vector after the last om
                om_tiles = []
                stc_list = []
                stc2_list = []
                for ht in range(HT):
                    stc_t = p_post.tile([128, ROWS], F16, tag='stc', bufs=3,
                                        name=f'stc_{ht}')
                    nc.vector.scalar_tensor_tensor(
                        stc_t, cab_tiles[ht], s_eff[:, ht:ht + 1],
                        st_tiles[ht], MULT, ADD)
                    stc_list.append(stc_t)
                    om_t = p_post.tile([128, ROWS], F16, tag='om', bufs=8,
                                       name=f'om_{ht}')
                    nc.vector.tensor_mul(om_t, q_tiles[ht][0], stc_t)
                    om_tiles.append(om_t)
                    stc2_t = p_post.tile([128, ROWS], F16, tag='stc2',
                                         bufs=8, name=f'stc2_{ht}')
                    nc.scalar.activation(stc2_t, stc_t, SQUARE)
                    stc2_list.append(stc2_t)
                # sq overwrites the q2 tile in place (saves SBUF)
                sq_tiles = []
                for ht in range(HT):
                    q2_t = q_tiles[ht][1]
                    nc.vector.tensor_mul(q2_t, q2_t, stc2_list[ht])
                    sq_tiles.append(q2_t)

                # ---- projection: nd0 7-wide waves (om-availability order),
                # drains release PSUM banks immediately (no rstd
                # dependency), then the ss matmuls run as sq arrives, then
                # nd1 solid; scales+stores happen once rstd lands ----------
                ss_ps = p_ssp.tile([33, 512], F32, tag='ssps', name='ss')
                pj0 = [p_ps.tile([128, 512], F32, tag='ps', name=f'pj0_{mr}')
                       for mr in range(7)]

                def pj_wave(pjs, nd, kt):
                    for mr, pj in enumerate(pjs):
                        nc.tensor.matmul(
                            pj,
                            lhsT=om_tiles[kt][:, mr * 128:(mr + 1) * 128],
                            rhs=opw_sb[:, kt, nd * 512:(nd + 1) * 512],
                            start=(kt == 0), stop=(kt == HT - 1))

                def ss_step(kt):
                    for nr in range(NR):
                        nc.tensor.matmul(
                            ss_ps[32 * nr:32 * nr + 1, :], lhsT=ones_sb,
                            rhs=sq_tiles[kt][:, nr * 512:(nr + 1) * 512],
                            start=(kt == 0), stop=(kt == HT - 1),
                            skip_group_check=True)

                raws = []

                def drain(pj, nd, mr):
                    raw_t = p_post.tile([128, 512], F16, tag='raw', bufs=16,
                                        name=f'raw_{nd}_{mr}')
                    if mr % 2 == 1:
                        nc.scalar.activation(raw_t, pj, COPY)
                    else:
                        nc.vector.tensor_copy(raw_t, pj)
                    raws.append((nd, mr, raw_t))

                for kt in range(HT):
                    pj_wave(pj0, 0, kt)
                for mr in range(7):
                    drain(pj0[mr], 0, mr)
                pj7 = p_ps.tile([128, 512], F32, tag='ps', name='pj0_7')
                for kt in range(HT):
                    nc.tensor.matmul(
                        pj7, lhsT=om_tiles[kt][:, 7 * 128:8 * 128],
                        rhs=opw_sb[:, kt, 0:512],
                        start=(kt == 0), stop=(kt == HT - 1))
                drain(pj7, 0, 7)

                # sum-of-squares right after the sq block, before nd1, so
                # rstd is ready while nd1 is still on the PE
                for kt in range(HT):
                    ss_step(kt)

                # rstd = sqrt(HID/(ss + HID*eps)) with the 4096x sq scale
                # folded in; DRAM-bounce transpose puts it on 128 lanes
                ss_sb = p_res.tile([33, 512], F32)
                nc.vector.tensor_copy(ss_sb[0:1, :], ss_ps[0:1, :])
                nc.vector.tensor_copy(ss_sb[32:33, :], ss_ps[32:33, :])
                r_dram = p_dram.tile([1, ROWS], F32)
                nc.sync.dma_start(out=r_dram[:, 0:512], in_=ss_sb[0:1, :])
                nc.sync.dma_start(out=r_dram[:, 512:1024],
                                  in_=ss_sb[32:33, :])
                ss_pc = p_res.tile([128, DT], F32)
                nc.sync.dma_start(
                    out=ss_pc,
                    in_=r_dram.rearrange('one (m p) -> p (one m)', p=128))
                msb = p_res.tile([128, DT], F32)
                nc.vector.tensor_scalar_add(msb, ss_pc, HID_EPS_S)
                rec = p_res.tile([128, DT], F32)
                nc.vector.reciprocal(rec, msb)
                rstd_pc = p_res.tile([128, DT], F32)
                nc.scalar.activation(rstd_pc, rec, SQRT, scale=HID_S)

                # nd=1 groups (PSUM banks recycle off the drains alone)
                for mr in range(DT):
                    pj = p_ps.tile([128, 512], F32, tag='ps',
                                   name=f'pj1_{mr}')
                    for kt in range(HT):
                        nc.tensor.matmul(
                            pj, lhsT=om_tiles[kt][:, mr * 128:(mr + 1) * 128],
                            rhs=opw_sb[:, kt, 512:1024],
                            start=(kt == 0), stop=(kt == HT - 1))
                    drain(pj, 1, mr)

                # scale + store once rstd is available
                for i, (nd, mr, raw_t) in enumerate(raws):
                    fin_t = p_post.tile([128, 512], F16, tag='fin', bufs=4,
                                        name=f'fin_{nd}_{mr}')
                    if i % 2 == 0:
                        nc.vector.tensor_scalar_mul(fin_t, raw_t,
                                                    rstd_pc[:, mr:mr + 1])
                    else:
                        nc.scalar.activation(fin_t, raw_t, COPY,
                                             scale=rstd_pc[:, mr:mr + 1])
                    nc.sync.dma_start(
                        out=out_d[mr * 128:(mr + 1) * 128,
                                  nd * 512:(nd + 1) * 512],
                        in_=fin_t)

    nc.finalize()
    return nc


def _softmax(x):
    e = np.exp(x - x.max())
    return e / e.sum()


def _lin128(W, nt, dtype=F16NP):
    """[nt*128, free] f32 -> [128, nt, free] (partition-major linear)."""
    free = W.shape[1]
    return np.ascontiguousarray(
        W.reshape(nt, 128, free).transpose(1, 0, 2)).astype(dtype)


def _host_prep(inputs):
    """Build the 8 per-core input maps."""
    x = np.asarray(inputs['x'], np.float32)
    top_k = int(inputs['top_k'])

    def bank(U, V, logits, dtype=F16NP, scale=1.0):
        w = _softmax(np.asarray(logits, np.float32))
        idx = np.argsort(-w)[:top_k]
        vals = w[idx]
        vals = vals / vals.sum()
        U = np.asarray(U, np.float32)[idx]              # [k, D, R]
        V = np.asarray(V, np.float32)[idx]              # [k, R, HID]
        ucat = np.transpose(U, (1, 0, 2)).reshape(D, top_k * RANK)
        vcat = (V * vals[:, None, None]).reshape(top_k * RANK, HID)
        return _lin128(ucat @ vcat * scale, DT, dtype)  # [128, DT, HID]

    wg8 = bank(inputs['v_U'], inputs['v_V'], inputs['gate_logits'],
               F8NP, S_G)
    wk = bank(inputs['k_U'], inputs['k_V'], inputs['k_logits'])
    wv = bank(inputs['v_U'], inputs['v_V'], inputs['v_logits'])
    wq = bank(inputs['q_U'], inputs['q_V'], inputs['q_logits'])

    ogw_f = np.ascontiguousarray(
        np.asarray(inputs['out_gate_w'], np.float32).T)           # [D, HID]
    og8 = _lin128(ogw_f * S_OG, DT, F8NP)                         # [128,8,HID]
    opw = _lin128(np.ascontiguousarray(
        (np.asarray(inputs['out_proj_w'], np.float32)
         * np.asarray(inputs['rms_w'], np.float32)[None, :]).T), HT)

    # decay on host (f32): z = x @ decay_w.T + b ; ld = -softplus(z)
    dw = np.asarray(inputs['decay_w'], np.float32)        # [H, D]
    db = np.asarray(inputs['decay_b'], np.float32)        # [H]
    z = np.einsum('bsd,hd->bsh', x, dw) + db              # [B, S, H]
    ld = -np.logaddexp(0.0, z)                            # log a
    a = np.exp(ld)                                        # sigmoid(-z)

    in_maps = []
    for c in range(NCORES):
        b, s2 = c // 2, c % 2
        sl = slice(s2 * ROWS, (s2 + 1) * ROWS)
        xc = np.ascontiguousarray(x[b, sl].T)
        xt = _lin128(xc, DT)                                        # fp16
        x8 = _lin128(xc, DT, F8NP)
        a_c = np.ascontiguousarray(a[b, sl].T).astype(np.float32)   # [H, ROWS]
        ca_c = np.ascontiguousarray(
            np.exp(np.cumsum(ld[b, sl], axis=0)).T).astype(F16NP)   # [H, ROWS]
        m_first = 1.0 if s2 == 0 else 0.0
        in_maps.append({
            'xt': xt, 'x8': x8, 'wg8': wg8,
            'wk': wk, 'wv': wv, 'wq': wq,
            'og8': og8, 'opw': opw,
            'a_t': a_c, 'ca_t': ca_c,
            'mc': np.full((128, 1), m_first, np.float32),
            'ma': np.full((128, 1), 1.0 - m_first, np.float32),
        })
    return in_maps


def kernel(**inputs) -> np.ndarray:
    from concourse.bass_utils import run_bass_kernel_spmd

    if 'nc' not in _BUILT:
        _BUILT['nc'] = _build()
    nc = _BUILT['nc']

    in_maps = _host_prep(inputs)
    res = run_bass_kernel_spmd(nc, in_maps, core_ids=list(range(NCORES)))

    out = np.empty((B, S, D), np.float32)
    for c in range(NCORES):
        b, s2 = c // 2, c % 2
        out[b, s2 * ROWS:(s2 + 1) * ROWS, :] = \
            np.asarray(res.results[c]['out'], dtype=np.float32)
    return out
